# revision 19
# baseline (speedup 1.0000x reference)
"""KAN embeddings Bass kernel for Trainium2, 8-core data-parallel over batch.

out[b,i,d] = silu(x[b,i]) * base_w[i,d] + sum_g exp(-0.5(x[b,i]-grid[g])^2) * gp_w[i,g,d]

Strategy per core (batch shard of 256 rows, 2 chunks of 128 partitions):
  - Fold base branch into the einsum: row 64 of the "feature" stationary is
    silu(x) and row 64 of the weights is base_w. The contraction is padded
    to K=128 with zero weight rows: full-height stationaries engage the
    PE's fast-weight-load path (measured 230 vs 427 ns per N=512 matmul).
  - RBF features via exp(-0.5 x^2 + g*x - 0.5 g^2): the argument is built
    by a K=5 all-bf16 broadcast-matmul (1 cyc/row) using a double-bf16
    (hi+lo) split of both -x^2/2 and x (argument error ~3e-5, far below
    the bf16 output quantization), then one ACT pass exp(in + bias[g])
    with per-partition bias -0.5 g^2, written as bf16 feature tiles.
  - Main matmuls fully bf16 (features + weights), 1 cyc/row: stationary =
    feat block cols (128,128), moving = weights (128,512), PSUM f32.
  - Weights are host-transposed to (G+1, NF*D) so weight loads are 8 KiB
    contiguous per partition (full DMA-engine rate).
  - PSUM -> SBUF copies (only ACT and DVE can read PSUM) drain 2-bank
    128x1024 tiles, downcast to bf16, split 6:10 ACT:DVE; output DMAs are
    bf16 (8 KiB/partition contiguous), halving HBM write traffic vs f32,
    and are issued per half-stage so they overlap the remaining copies.
    All loads ride the sync HWDGE ring ahead of the output stores
    (block loads one block early, weight loads two stages early); SWDGE
    (gpsimd dma) is avoided for anything latency-critical. Host upcasts
    the bf16 output to f32.
"""

import numpy as np

B, NF, G, D = 2048, 256, 64, 512
NCORES = 8
BL = B // NCORES          # 256 batch rows per core
NBLK = 16                 # features per block
NW = 8                    # features per weight chunk / output stage
NGRP = NBLK // 2          # bcast-MM groups per block (2 feats x 256 b = N 512)
KF = 5                    # feature-matmul contraction: mx2_hi,mx2_lo,x_hi,x_hi,x_lo

# PSUM->SBUF copy engine schedule. Only ACT and DVE can read PSUM on TRN2
# (GPSIMD/Pool cannot). Copies are issued per PAIR of matmuls (a 2-bank
# 128x1024 PSUM tile drained by one instruction); 7xACT/9xDVE per 16 plus
# the exp pass on ACT keeps both engines ~180us, under the ~235us DMA floor.
_COPY_PAT = "ADDADDADADDADDAD"

_cache = {}


def _build():
    import concourse.bass as bass
    from concourse import mybir
    from concourse import tile

    f32 = mybir.dt.float32
    bf16 = mybir.dt.bfloat16
    AF = mybir.ActivationFunctionType

    nc = bass.Bass()
    x5 = nc.declare_dram_parameter("x5", [KF, NF * BL], bf16, isOutput=False)
    xs = nc.declare_dram_parameter("xs", [1, NF * BL], bf16, isOutput=False)
    wcatT = nc.declare_dram_parameter("wcatT", [G + 1, NF * D], bf16, isOutput=False)
    s5 = nc.declare_dram_parameter("s5", [KF, G], bf16, isOutput=False)
    nb2 = nc.declare_dram_parameter("nb2", [G, 1], f32, isOutput=False)
    out = nc.declare_dram_parameter("out", [BL, NF, D], bf16, isOutput=True)

    nblocks = NF // NBLK
    nstages = nblocks * (NBLK // NW)

    with tile.TileContext(nc) as tc:
        with (
            tc.tile_pool(name="const", bufs=1) as constp,
            tc.tile_pool(name="x5p", bufs=3) as x5p,
            tc.tile_pool(name="stage", bufs=8) as stagep,
            tc.tile_pool(name="pt", bufs=2, space="PSUM") as ptp,
            tc.tile_pool(name="po", bufs=3, space="PSUM") as pop,
        ):
            s5_t = constp.tile([KF, G], bf16)
            nc.sync.dma_start(out=s5_t[:, :], in_=s5[:, :])
            nb2_t = constp.tile([G, 1], f32)
            nc.sync.dma_start(out=nb2_t[:, :], in_=nb2[:, :])

            # Persistent K=128-padded operand slots. FWL (fast weight load)
            # only engages for full-height 128-row stationaries -- measured
            # 230 ns vs 427 ns per N=512 matmul. Rows G+1..127 are zeroed
            # once; the per-stage DMAs only rewrite rows 0..G, so the
            # padding costs no HBM traffic. Manual slot rotation; Tile's
            # dependency tracking turns reuse into the usual WAR waits.
            fb_slots = [
                constp.tile([128, NBLK * BL], bf16, name=f"fbs{i}")
                for i in range(4)
            ]
            wt_slots = [
                constp.tile([128, NW * D], bf16, name=f"wts{i}")
                for i in range(4)
            ]
            # engine APs must start on a 32-partition boundary: zero rows
            # 64..127; row 64 (silu / base_w) is rewritten by DMA afterwards.
            # wt slots go first on gpsimd and fb slots on the (startup-idle)
            # DVE so the first weight DMA (WAW on row 64) is not gated
            # behind ~14us of serial memsets.
            for t in wt_slots:
                nc.gpsimd.memset(t[G:128, :], 0.0)
            for t in fb_slots:
                nc.vector.memset(t[G:128, :], 0.0)

            # All loads ride the sync HWDGE ring AHEAD of the output stores
            # they must beat (software pipelining): block loads lead by one
            # block, weight loads by two stages. SWDGE (gpsimd) is avoided:
            # its Q7 descriptor generation stalls against concurrent engine
            # activity on the shared port.
            def issue_block_loads(b):
                if b >= nblocks:
                    return None
                x5_t = x5p.tile([KF, NBLK * BL], bf16)
                nc.sync.dma_start(
                    out=x5_t[:, :], in_=x5[0:KF, b * NBLK * BL:(b + 1) * NBLK * BL]
                )
                fb = fb_slots[b % 4]
                # silu row straight from DRAM into partition 64
                nc.sync.dma_start(
                    out=fb[G:G + 1, :],
                    in_=xs[0:1, b * NBLK * BL:(b + 1) * NBLK * BL],
                )
                return x5_t, fb

            def issue_wt_load(s):
                if s >= nstages:
                    return None
                iw = s * NW
                wt = wt_slots[s % 4]
                nc.sync.dma_start(
                    out=wt[0:G + 1, :], in_=wcatT[0:G + 1, iw * D:(iw + NW) * D]
                )
                return wt

            blk_tiles = {0: issue_block_loads(0)}
            wts = {0: issue_wt_load(0), 1: issue_wt_load(1)}

            ncopy = 0
            for blk in range(nblocks):
                blk_tiles[blk + 1] = issue_block_loads(blk + 1)
                x5_t, fb = blk_tiles.pop(blk)
                # feature computation: 8 groups of 2 features
                for g2 in range(NGRP):
                    pt = ptp.tile([G, 512], f32)
                    nc.tensor.matmul(
                        pt[:, :],
                        s5_t[:, :],
                        x5_t[0:KF, g2 * 512:(g2 + 1) * 512],
                        start=True,
                        stop=True,
                    )
                    nc.scalar.activation(
                        fb[0:G, g2 * 512:(g2 + 1) * 512],
                        pt[:, :],
                        AF.Exp,
                        bias=nb2_t[:, :],
                        scale=1.0,
                    )
                # main matmuls in two weight chunks of NW features
                for wc in range(NBLK // NW):
                    s = blk * (NBLK // NW) + wc
                    wts[s + 2] = issue_wt_load(s + 2)
                    wt = wts.pop(s)
                    iw = blk * NBLK + wc * NW
                    st0 = stagep.tile([128, NW * D], bf16, tag="stage")
                    st1 = stagep.tile([128, NW * D], bf16, tag="stage")
                    sts = (st0, st1)
                    for j2 in range(NW // 2):
                        for c in range(2):
                            po = pop.tile([128, 2 * D], f32)
                            for h in range(2):
                                j = 2 * j2 + h
                                i_loc = wc * NW + j
                                nc.tensor.matmul(
                                    po[:, h * D:(h + 1) * D],
                                    fb[0:128,
                                       i_loc * BL + c * 128:
                                       i_loc * BL + c * 128 + 128],
                                    wt[0:128, j * D:(j + 1) * D],
                                    start=True,
                                    stop=True,
                                )
                            dst = sts[c][:, 2 * j2 * D:(2 * j2 + 2) * D]
                            eng = _COPY_PAT[ncopy % len(_COPY_PAT)]
                            ncopy += 1
                            if eng == "A":
                                nc.scalar.copy(dst, po[:, :])
                            else:
                                nc.vector.tensor_scalar_mul(dst, po[:, :], 1.0)
                            # drain each half-stage as soon as its two
                            # pair-copies land: transfers overlap the
                            # remaining copies instead of waiting stage-end
                            if j2 % 2 == 1:
                                hw0 = (j2 - 1) * 2
                                nc.sync.dma_start(
                                    out=out[c * 128:(c + 1) * 128,
                                            iw + hw0:iw + hw0 + 4, :],
                                    in_=sts[c][:, hw0 * D:(hw0 + 4) * D],
                                )

    _split_multi_waits(nc)
    return nc


def _split_multi_waits(nc):
    """Walrus TPB instruction structs accept a single sync wait. Hoist all
    but the last wait of any instruction onto same-engine NOPs inserted
    immediately before it (a wait executes before the instruction either
    way, so this is semantically identical)."""
    import dataclasses
    import concourse.bass as bass
    import concourse.mybir as mybir

    tpl = bass.Bass().sync.nop().ins
    k = 0
    for blk in nc.m.functions[0].blocks:
        out_insts = []
        for inst in blk.instructions:
            si = getattr(inst, "sync_info", None)
            if si is not None and len(si.on_wait) > 1:
                for w in si.on_wait[:-1]:
                    out_insts.append(
                        dataclasses.replace(
                            tpl,
                            name=f"nop-w{k}",
                            engine=inst.engine,
                            sync_info=mybir.SyncInfo(on_wait=[w], on_update=[]),
                        )
                    )
                    k += 1
                inst.sync_info = dataclasses.replace(si, on_wait=si.on_wait[-1:])
            out_insts.append(inst)
        blk.instructions[:] = out_insts


def _hi_lo(v, bf16):
    """Double-bf16 split: v ~= hi + lo with |err| <~ |v| * 2^-17."""
    hi = v.astype(bf16)
    lo = (v - hi.astype(np.float32)).astype(bf16)
    return hi, lo


def _prep_inputs(x, base_weight, gp_weight, grid):
    import ml_dtypes

    bf16 = ml_dtypes.bfloat16
    x = np.ascontiguousarray(np.asarray(x, np.float32))
    base_weight = np.asarray(base_weight, np.float32)
    gp_weight = np.asarray(gp_weight, np.float32)
    grid = np.asarray(grid, np.float32)

    # (G+1, NF, D) bf16, g-major so weight loads are contiguous per grid row
    wcatT = np.ascontiguousarray(
        np.concatenate([gp_weight, base_weight[:, None, :]], axis=1)
        .transpose(1, 0, 2)
        .reshape(G + 1, NF * D)
        .astype(bf16)
    )
    g_hi, g_lo = _hi_lo(grid, bf16)
    ones = np.ones(G, bf16)
    # rows pair with moving rows [mx2_hi, mx2_lo, x_hi, x_hi, x_lo]
    s5 = np.ascontiguousarray(np.stack([ones, ones, g_hi, g_lo, g_hi]))
    nb2 = np.ascontiguousarray((-0.5 * grid * grid).reshape(G, 1))

    in_maps = []
    for c in range(NCORES):
        xT = np.ascontiguousarray(x[c * BL:(c + 1) * BL, :].T)  # (NF, BL)
        mx2 = (-0.5 * xT * xT).ravel()
        xr = xT.ravel()
        mx2_hi, mx2_lo = _hi_lo(mx2, bf16)
        x_hi, x_lo = _hi_lo(xr, bf16)
        x5 = np.ascontiguousarray(np.stack([mx2_hi, mx2_lo, x_hi, x_hi, x_lo]))
        xs = np.ascontiguousarray(
            (xT / (1.0 + np.exp(-xT))).ravel().astype(bf16).reshape(1, NF * BL)
        )  # silu, bf16
        in_maps.append(
            {"x5": x5, "xs": xs, "wcatT": wcatT, "s5": s5, "nb2": nb2}
        )
    return in_maps


def _run(in_maps, **kw):
    from concourse.bass_utils import run_bass_kernel_spmd

    if "nc" not in _cache:
        _cache["nc"] = _build()
    return run_bass_kernel_spmd(_cache["nc"], in_maps, list(range(NCORES)), **kw)


def kernel(x, base_weight, gp_weight, grid):
    in_maps = _prep_inputs(x, base_weight, gp_weight, grid)
    res = _run(in_maps)
    return np.concatenate(
        [np.asarray(r["out"]).astype(np.float32) for r in res.results], axis=0
    )


# revision 20
# speedup vs baseline: 1.0116x; 1.0116x over previous
"""KAN embeddings Bass kernel for Trainium2, 8-core data-parallel over batch.

out[b,i,d] = silu(x[b,i]) * base_w[i,d] + sum_g exp(-0.5(x[b,i]-grid[g])^2) * gp_w[i,g,d]

Strategy per core (batch shard of 256 rows, 2 chunks of 128 partitions):
  - Fold base branch into the einsum: row 64 of the "feature" stationary is
    silu(x) and row 64 of the weights is base_w. The contraction is padded
    to K=128 with zero weight rows: full-height stationaries engage the
    PE's fast-weight-load path (measured 230 vs 427 ns per N=512 matmul).
  - RBF features via exp(-0.5 x^2 + g*x - 0.5 g^2): the argument is built
    by a K=5 all-bf16 broadcast-matmul (1 cyc/row) using a double-bf16
    (hi+lo) split of both -x^2/2 and x (argument error ~3e-5, far below
    the bf16 output quantization), then one ACT pass exp(in + bias[g])
    with per-partition bias -0.5 g^2, written as bf16 feature tiles.
  - Main matmuls fully bf16 (features + weights), 1 cyc/row: stationary =
    feat block cols (128,128), moving = weights (128,512), PSUM f32.
  - Weights are host-transposed to (G+1, NF*D) so weight loads are 8 KiB
    contiguous per partition (full DMA-engine rate).
  - PSUM -> SBUF copies (only ACT and DVE can read PSUM) drain 2-bank
    128x1024 tiles, downcast to bf16, split 6:10 ACT:DVE; output DMAs are
    bf16 (8 KiB/partition contiguous), halving HBM write traffic vs f32,
    and are issued per half-stage so they overlap the remaining copies.
    All loads ride the sync HWDGE ring ahead of the output stores
    (block loads one block early, weight loads two stages early); SWDGE
    (gpsimd dma) is avoided for anything latency-critical. Host upcasts
    the bf16 output to f32.
"""

import numpy as np

B, NF, G, D = 2048, 256, 64, 512
NCORES = 8
BL = B // NCORES          # 256 batch rows per core
NBLK = 16                 # features per block
NW = 8                    # features per weight chunk / output stage
NGRP = NBLK // 2          # bcast-MM groups per block (2 feats x 256 b = N 512)
KF = 5                    # feature-matmul contraction: mx2_hi,mx2_lo,x_hi,x_hi,x_lo

# PSUM->SBUF copy engine schedule. Only ACT and DVE can read PSUM on TRN2
# (GPSIMD/Pool cannot). Copies are issued per PAIR of matmuls (a 2-bank
# 128x1024 PSUM tile drained by one instruction); 6xACT/10xDVE per 16 plus
# the exp pass on ACT keeps both engines ~180us, under the ~235us DMA floor.
_COPY_PAT = "ADDADDADADDADDAD"

_cache = {}


def _build():
    import concourse.bass as bass
    from concourse import mybir
    from concourse import tile

    f32 = mybir.dt.float32
    bf16 = mybir.dt.bfloat16
    AF = mybir.ActivationFunctionType

    nc = bass.Bass()
    x5 = nc.declare_dram_parameter("x5", [KF, NF * BL], bf16, isOutput=False)
    xs = nc.declare_dram_parameter("xs", [1, NF * BL], bf16, isOutput=False)
    wcatT = nc.declare_dram_parameter("wcatT", [G + 1, NF * D], bf16, isOutput=False)
    s5 = nc.declare_dram_parameter("s5", [KF, G], bf16, isOutput=False)
    nb2 = nc.declare_dram_parameter("nb2", [G, 1], f32, isOutput=False)
    out = nc.declare_dram_parameter("out", [BL, NF, D], bf16, isOutput=True)

    nblocks = NF // NBLK
    nstages = nblocks * (NBLK // NW)

    with tile.TileContext(nc) as tc:
        with (
            tc.tile_pool(name="const", bufs=1) as constp,
            tc.tile_pool(name="x5p", bufs=2) as x5p,
            tc.tile_pool(name="stage", bufs=6) as stagep,
            tc.tile_pool(name="pt", bufs=2, space="PSUM") as ptp,
            tc.tile_pool(name="po", bufs=3, space="PSUM") as pop,
        ):
            s5_t = constp.tile([KF, G], bf16)
            nc.sync.dma_start(out=s5_t[:, :], in_=s5[:, :])
            nb2_t = constp.tile([G, 1], f32)
            nc.sync.dma_start(out=nb2_t[:, :], in_=nb2[:, :])

            # Persistent K=128-padded operand slots. FWL (fast weight load)
            # only engages for full-height 128-row stationaries -- measured
            # 230 ns vs 427 ns per N=512 matmul. Rows G+1..127 are zeroed
            # once; the per-stage DMAs only rewrite rows 0..G, so the
            # padding costs no HBM traffic. Manual slot rotation; Tile's
            # dependency tracking turns reuse into the usual WAR waits.
            fb_slots = [
                constp.tile([128, NBLK * BL], bf16, name=f"fbs{i}")
                for i in range(3)
            ]
            wt_slots = [
                constp.tile([128, NW * D], bf16, name=f"wts{i}")
                for i in range(4)
            ]
            # engine APs must start on a 32-partition boundary: zero rows
            # 64..127; row 64 (silu / base_w) is rewritten by DMA afterwards.
            # wt slots go first on gpsimd and fb slots on the (startup-idle)
            # DVE so the first weight DMA (WAW on row 64) is not gated
            # behind ~14us of serial memsets.
            for t in wt_slots:
                nc.gpsimd.memset(t[G:128, :], 0.0)
            for t in fb_slots:
                nc.vector.memset(t[G:128, :], 0.0)

            # All loads ride the sync HWDGE ring AHEAD of the output stores
            # they must beat (software pipelining): block loads lead by one
            # block, weight loads by two stages. SWDGE (gpsimd) is avoided:
            # its Q7 descriptor generation stalls against concurrent engine
            # activity on the shared port.
            def issue_block_loads(b):
                if b >= nblocks:
                    return None
                x5_t = x5p.tile([KF, NBLK * BL], bf16)
                nc.sync.dma_start(
                    out=x5_t[:, :], in_=x5[0:KF, b * NBLK * BL:(b + 1) * NBLK * BL]
                )
                fb = fb_slots[b % 3]
                # silu row straight from DRAM into partition 64
                nc.sync.dma_start(
                    out=fb[G:G + 1, :],
                    in_=xs[0:1, b * NBLK * BL:(b + 1) * NBLK * BL],
                )
                return x5_t, fb

            def issue_wt_load(s):
                if s >= nstages:
                    return None
                iw = s * NW
                wt = wt_slots[s % 4]
                nc.sync.dma_start(
                    out=wt[0:G + 1, :], in_=wcatT[0:G + 1, iw * D:(iw + NW) * D]
                )
                return wt

            blk_tiles = {0: issue_block_loads(0)}
            wts = {0: issue_wt_load(0), 1: issue_wt_load(1)}

            ncopy = 0
            for blk in range(nblocks):
                blk_tiles[blk + 1] = issue_block_loads(blk + 1)
                x5_t, fb = blk_tiles.pop(blk)
                # feature computation: 8 groups of 2 features
                for g2 in range(NGRP):
                    pt = ptp.tile([G, 512], f32)
                    nc.tensor.matmul(
                        pt[:, :],
                        s5_t[:, :],
                        x5_t[0:KF, g2 * 512:(g2 + 1) * 512],
                        start=True,
                        stop=True,
                    )
                    nc.scalar.activation(
                        fb[0:G, g2 * 512:(g2 + 1) * 512],
                        pt[:, :],
                        AF.Exp,
                        bias=nb2_t[:, :],
                        scale=1.0,
                    )
                # main matmuls in two weight chunks of NW features
                for wc in range(NBLK // NW):
                    s = blk * (NBLK // NW) + wc
                    wts[s + 2] = issue_wt_load(s + 2)
                    wt = wts.pop(s)
                    iw = blk * NBLK + wc * NW
                    st0 = stagep.tile([128, NW * D], bf16, tag="stage")
                    st1 = stagep.tile([128, NW * D], bf16, tag="stage")
                    sts = (st0, st1)
                    for j2 in range(NW // 2):
                        for c in range(2):
                            po = pop.tile([128, 2 * D], f32)
                            for h in range(2):
                                j = 2 * j2 + h
                                i_loc = wc * NW + j
                                nc.tensor.matmul(
                                    po[:, h * D:(h + 1) * D],
                                    fb[0:128,
                                       i_loc * BL + c * 128:
                                       i_loc * BL + c * 128 + 128],
                                    wt[0:128, j * D:(j + 1) * D],
                                    start=True,
                                    stop=True,
                                )
                            dst = sts[c][:, 2 * j2 * D:(2 * j2 + 2) * D]
                            eng = _COPY_PAT[ncopy % len(_COPY_PAT)]
                            ncopy += 1
                            if eng == "A":
                                nc.scalar.copy(dst, po[:, :])
                            else:
                                nc.vector.tensor_scalar_mul(dst, po[:, :], 1.0)
                            # drain each half-stage as soon as its two
                            # pair-copies land: transfers overlap the
                            # remaining copies instead of waiting stage-end
                            if j2 % 2 == 1:
                                hw0 = (j2 - 1) * 2
                                nc.sync.dma_start(
                                    out=out[c * 128:(c + 1) * 128,
                                            iw + hw0:iw + hw0 + 4, :],
                                    in_=sts[c][:, hw0 * D:(hw0 + 4) * D],
                                )

    _split_multi_waits(nc)
    return nc


def _split_multi_waits(nc):
    """Walrus TPB instruction structs accept a single sync wait. Hoist all
    but the last wait of any instruction onto same-engine NOPs inserted
    immediately before it (a wait executes before the instruction either
    way, so this is semantically identical)."""
    import dataclasses
    import concourse.bass as bass
    import concourse.mybir as mybir

    tpl = bass.Bass().sync.nop().ins
    k = 0
    for blk in nc.m.functions[0].blocks:
        out_insts = []
        for inst in blk.instructions:
            si = getattr(inst, "sync_info", None)
            if si is not None and len(si.on_wait) > 1:
                for w in si.on_wait[:-1]:
                    out_insts.append(
                        dataclasses.replace(
                            tpl,
                            name=f"nop-w{k}",
                            engine=inst.engine,
                            sync_info=mybir.SyncInfo(on_wait=[w], on_update=[]),
                        )
                    )
                    k += 1
                inst.sync_info = dataclasses.replace(si, on_wait=si.on_wait[-1:])
            out_insts.append(inst)
        blk.instructions[:] = out_insts


def _hi_lo(v, bf16):
    """Double-bf16 split: v ~= hi + lo with |err| <~ |v| * 2^-17."""
    hi = v.astype(bf16)
    lo = (v - hi.astype(np.float32)).astype(bf16)
    return hi, lo


def _prep_inputs(x, base_weight, gp_weight, grid):
    import ml_dtypes

    bf16 = ml_dtypes.bfloat16
    x = np.ascontiguousarray(np.asarray(x, np.float32))
    base_weight = np.asarray(base_weight, np.float32)
    gp_weight = np.asarray(gp_weight, np.float32)
    grid = np.asarray(grid, np.float32)

    # (G+1, NF, D) bf16, g-major so weight loads are contiguous per grid row
    wcatT = np.ascontiguousarray(
        np.concatenate([gp_weight, base_weight[:, None, :]], axis=1)
        .transpose(1, 0, 2)
        .reshape(G + 1, NF * D)
        .astype(bf16)
    )
    g_hi, g_lo = _hi_lo(grid, bf16)
    ones = np.ones(G, bf16)
    # rows pair with moving rows [mx2_hi, mx2_lo, x_hi, x_hi, x_lo]
    s5 = np.ascontiguousarray(np.stack([ones, ones, g_hi, g_lo, g_hi]))
    nb2 = np.ascontiguousarray((-0.5 * grid * grid).reshape(G, 1))

    in_maps = []
    for c in range(NCORES):
        xT = np.ascontiguousarray(x[c * BL:(c + 1) * BL, :].T)  # (NF, BL)
        mx2 = (-0.5 * xT * xT).ravel()
        xr = xT.ravel()
        mx2_hi, mx2_lo = _hi_lo(mx2, bf16)
        x_hi, x_lo = _hi_lo(xr, bf16)
        x5 = np.ascontiguousarray(np.stack([mx2_hi, mx2_lo, x_hi, x_hi, x_lo]))
        xs = np.ascontiguousarray(
            (xT / (1.0 + np.exp(-xT))).ravel().astype(bf16).reshape(1, NF * BL)
        )  # silu, bf16
        in_maps.append(
            {"x5": x5, "xs": xs, "wcatT": wcatT, "s5": s5, "nb2": nb2}
        )
    return in_maps


def _run(in_maps, **kw):
    from concourse.bass_utils import run_bass_kernel_spmd

    if "nc" not in _cache:
        _cache["nc"] = _build()
    return run_bass_kernel_spmd(_cache["nc"], in_maps, list(range(NCORES)), **kw)


def kernel(x, base_weight, gp_weight, grid):
    in_maps = _prep_inputs(x, base_weight, gp_weight, grid)
    res = _run(in_maps)
    return np.concatenate(
        [np.asarray(r["out"]).astype(np.float32) for r in res.results], axis=0
    )


# revision 21
# speedup vs baseline: 1.0156x; 1.0040x over previous
"""KAN embeddings Bass kernel for Trainium2, 8-core data-parallel over batch.

out[b,i,d] = silu(x[b,i]) * base_w[i,d] + sum_g exp(-0.5(x[b,i]-grid[g])^2) * gp_w[i,g,d]

Strategy per core (batch shard of 256 rows, 2 chunks of 128 partitions):
  - Fold base branch into the einsum: row 64 of the "feature" stationary is
    silu(x) and row 64 of the weights is base_w. The contraction is padded
    to K=128 with zero weight rows: full-height stationaries engage the
    PE's fast-weight-load path (measured 230 vs 427 ns per N=512 matmul).
  - RBF features via exp(-0.5 x^2 + g*x - 0.5 g^2): the argument is built
    by a K=5 all-bf16 broadcast-matmul (1 cyc/row) using a double-bf16
    (hi+lo) split of both -x^2/2 and x (argument error ~3e-5, far below
    the bf16 output quantization), then one ACT pass exp(in + bias[g])
    with per-partition bias -0.5 g^2, written as bf16 feature tiles.
  - Main matmuls fully bf16 (features + weights), 1 cyc/row: stationary =
    feat block cols (128,128), moving = weights (128,512), PSUM f32.
  - Weights are host-transposed to (G+1, NF*D) so weight loads are 8 KiB
    contiguous per partition (full DMA-engine rate).
  - PSUM -> SBUF copies (only ACT and DVE can read PSUM) drain 2-bank
    128x1024 tiles, downcast to bf16, split 6:10 ACT:DVE; output DMAs are
    bf16 (8 KiB/partition contiguous), halving HBM write traffic vs f32,
    and are issued per half-stage so they overlap the remaining copies.
    All loads ride the sync HWDGE ring ahead of the output stores
    (block loads one block early, weight loads two stages early); SWDGE
    (gpsimd dma) is avoided for anything latency-critical. Host upcasts
    the bf16 output to f32.
"""

import numpy as np

B, NF, G, D = 2048, 256, 64, 512
NCORES = 8
BL = B // NCORES          # 256 batch rows per core
NBLK = 16                 # features per block
NW = 8                    # features per weight chunk / output stage
NGRP = NBLK // 2          # bcast-MM groups per block (2 feats x 256 b = N 512)
KF = 5                    # feature-matmul contraction: mx2_hi,mx2_lo,x_hi,x_hi,x_lo

# PSUM->SBUF copy engine schedule. Only ACT and DVE can read PSUM on TRN2
# (GPSIMD/Pool cannot). Copies are issued per PAIR of matmuls (a 2-bank
# 128x1024 PSUM tile drained by one instruction); 6xACT/10xDVE per 16 plus
# the exp pass on ACT keeps both engines ~180us, under the ~235us DMA floor.
_COPY_PAT = "ADDADDADADDADDAD"

_cache = {}


def _build():
    import concourse.bass as bass
    from concourse import mybir
    from concourse import tile

    f32 = mybir.dt.float32
    bf16 = mybir.dt.bfloat16
    AF = mybir.ActivationFunctionType

    nc = bass.Bass()
    x5 = nc.declare_dram_parameter("x5", [KF, NF * BL], bf16, isOutput=False)
    xs = nc.declare_dram_parameter("xs", [1, NF * BL], bf16, isOutput=False)
    wcatT = nc.declare_dram_parameter("wcatT", [G + 1, NF * D], bf16, isOutput=False)
    s5 = nc.declare_dram_parameter("s5", [KF, G], bf16, isOutput=False)
    nb2 = nc.declare_dram_parameter("nb2", [G, 1], f32, isOutput=False)
    out = nc.declare_dram_parameter("out", [BL, NF, D], bf16, isOutput=True)

    nblocks = NF // NBLK
    nstages = nblocks * (NBLK // NW)

    with tile.TileContext(nc) as tc:
        with (
            tc.tile_pool(name="const", bufs=1) as constp,
            tc.tile_pool(name="x5p", bufs=2) as x5p,
            tc.tile_pool(name="stage", bufs=6) as stagep,
            tc.tile_pool(name="pt", bufs=2, space="PSUM") as ptp,
            tc.tile_pool(name="po", bufs=3, space="PSUM") as pop,
        ):
            s5_t = constp.tile([KF, G], bf16)
            nc.sync.dma_start(out=s5_t[:, :], in_=s5[:, :])
            nb2_t = constp.tile([G, 1], f32)
            nc.sync.dma_start(out=nb2_t[:, :], in_=nb2[:, :])

            # Persistent K=128-padded operand slots. FWL (fast weight load)
            # only engages for full-height 128-row stationaries -- measured
            # 230 ns vs 427 ns per N=512 matmul. Rows G+1..127 are zeroed
            # once; the per-stage DMAs only rewrite rows 0..G, so the
            # padding costs no HBM traffic. Manual slot rotation; Tile's
            # dependency tracking turns reuse into the usual WAR waits.
            fb_slots = [
                constp.tile([128, NBLK * BL], bf16, name=f"fbs{i}")
                for i in range(3)
            ]
            wt_slots = [
                constp.tile([128, NW * D], bf16, name=f"wts{i}")
                for i in range(4)
            ]
            # engine APs must start on a 32-partition boundary: zero rows
            # 64..127; row 64 (silu / base_w) is rewritten by DMA afterwards.
            # wt slots go first on gpsimd and fb slots on the (startup-idle)
            # DVE so the first weight DMA (WAW on row 64) is not gated
            # behind ~14us of serial memsets.
            for t in wt_slots:
                nc.gpsimd.memset(t[G:128, :], 0.0)
            for t in fb_slots:
                nc.vector.memset(t[G:128, :], 0.0)

            # All loads ride the sync HWDGE ring AHEAD of the output stores
            # they must beat (software pipelining): block loads lead by one
            # block, weight loads by two stages. SWDGE (gpsimd) is avoided:
            # its Q7 descriptor generation stalls against concurrent engine
            # activity on the shared port.
            def issue_block_loads(b):
                if b >= nblocks:
                    return None
                x5_t = x5p.tile([KF, NBLK * BL], bf16)
                nc.sync.dma_start(
                    out=x5_t[:, :], in_=x5[0:KF, b * NBLK * BL:(b + 1) * NBLK * BL]
                )
                fb = fb_slots[b % 3]
                # silu row straight from DRAM into partition 64
                nc.sync.dma_start(
                    out=fb[G:G + 1, :],
                    in_=xs[0:1, b * NBLK * BL:(b + 1) * NBLK * BL],
                )
                return x5_t, fb

            def issue_wt_load(s):
                if s >= nstages:
                    return None
                iw = s * NW
                wt = wt_slots[s % 4]
                nc.sync.dma_start(
                    out=wt[0:G + 1, :], in_=wcatT[0:G + 1, iw * D:(iw + NW) * D]
                )
                return wt

            blk_tiles = {0: issue_block_loads(0)}
            wts = {0: issue_wt_load(0), 1: issue_wt_load(1)}

            ncopy = 0
            for blk in range(nblocks):
                blk_tiles[blk + 1] = issue_block_loads(blk + 1)
                x5_t, fb = blk_tiles.pop(blk)
                # feature computation: 8 groups of 2 features
                for g2 in range(NGRP):
                    pt = ptp.tile([G, 512], f32)
                    nc.tensor.matmul(
                        pt[:, :],
                        s5_t[:, :],
                        x5_t[0:KF, g2 * 512:(g2 + 1) * 512],
                        start=True,
                        stop=True,
                    )
                    nc.scalar.activation(
                        fb[0:G, g2 * 512:(g2 + 1) * 512],
                        pt[:, :],
                        AF.Exp,
                        bias=nb2_t[:, :],
                        scale=1.0,
                    )
                # main matmuls in two weight chunks of NW features
                for wc in range(NBLK // NW):
                    s = blk * (NBLK // NW) + wc
                    wts[s + 2] = issue_wt_load(s + 2)
                    wt = wts.pop(s)
                    iw = blk * NBLK + wc * NW
                    st0 = stagep.tile([128, NW * D], bf16, tag="stage")
                    st1 = stagep.tile([128, NW * D], bf16, tag="stage")
                    sts = (st0, st1)
                    for j2 in range(NW // 2):
                        for c in range(2):
                            po = pop.tile([128, 2 * D], f32)
                            for h in range(2):
                                j = 2 * j2 + h
                                i_loc = wc * NW + j
                                nc.tensor.matmul(
                                    po[:, h * D:(h + 1) * D],
                                    fb[0:128,
                                       i_loc * BL + c * 128:
                                       i_loc * BL + c * 128 + 128],
                                    wt[0:128, j * D:(j + 1) * D],
                                    start=True,
                                    stop=True,
                                )
                            dst = sts[c][:, 2 * j2 * D:(2 * j2 + 2) * D]
                            eng = _COPY_PAT[ncopy % len(_COPY_PAT)]
                            ncopy += 1
                            if eng == "A":
                                nc.scalar.copy(dst, po[:, :])
                            else:
                                nc.vector.tensor_scalar_mul(dst, po[:, :], 1.0)
                            # drain each half-stage as soon as its two
                            # pair-copies land: transfers overlap the
                            # remaining copies instead of waiting stage-end.
                            # c=1 stores ride the gpsimd SWDGE queue: stores
                            # are latency-tolerant, so SWDGE's slower
                            # descriptor generation is harmless, and SP's
                            # serial wait-then-issue chain is halved.
                            if j2 % 2 == 1:
                                hw0 = (j2 - 1) * 2
                                eng_dma = nc.sync if c == 0 else nc.gpsimd
                                eng_dma.dma_start(
                                    out=out[c * 128:(c + 1) * 128,
                                            iw + hw0:iw + hw0 + 4, :],
                                    in_=sts[c][:, hw0 * D:(hw0 + 4) * D],
                                )

    _split_multi_waits(nc)
    return nc


def _split_multi_waits(nc):
    """Walrus TPB instruction structs accept a single sync wait. Hoist all
    but the last wait of any instruction onto same-engine NOPs inserted
    immediately before it (a wait executes before the instruction either
    way, so this is semantically identical)."""
    import dataclasses
    import concourse.bass as bass
    import concourse.mybir as mybir

    tpl = bass.Bass().sync.nop().ins
    k = 0
    for blk in nc.m.functions[0].blocks:
        out_insts = []
        for inst in blk.instructions:
            si = getattr(inst, "sync_info", None)
            if si is not None and len(si.on_wait) > 1:
                for w in si.on_wait[:-1]:
                    out_insts.append(
                        dataclasses.replace(
                            tpl,
                            name=f"nop-w{k}",
                            engine=inst.engine,
                            sync_info=mybir.SyncInfo(on_wait=[w], on_update=[]),
                        )
                    )
                    k += 1
                inst.sync_info = dataclasses.replace(si, on_wait=si.on_wait[-1:])
            out_insts.append(inst)
        blk.instructions[:] = out_insts


def _hi_lo(v, bf16):
    """Double-bf16 split: v ~= hi + lo with |err| <~ |v| * 2^-17."""
    hi = v.astype(bf16)
    lo = (v - hi.astype(np.float32)).astype(bf16)
    return hi, lo


def _prep_inputs(x, base_weight, gp_weight, grid):
    import ml_dtypes

    bf16 = ml_dtypes.bfloat16
    x = np.ascontiguousarray(np.asarray(x, np.float32))
    base_weight = np.asarray(base_weight, np.float32)
    gp_weight = np.asarray(gp_weight, np.float32)
    grid = np.asarray(grid, np.float32)

    # (G+1, NF, D) bf16, g-major so weight loads are contiguous per grid row
    wcatT = np.ascontiguousarray(
        np.concatenate([gp_weight, base_weight[:, None, :]], axis=1)
        .transpose(1, 0, 2)
        .reshape(G + 1, NF * D)
        .astype(bf16)
    )
    g_hi, g_lo = _hi_lo(grid, bf16)
    ones = np.ones(G, bf16)
    # rows pair with moving rows [mx2_hi, mx2_lo, x_hi, x_hi, x_lo]
    s5 = np.ascontiguousarray(np.stack([ones, ones, g_hi, g_lo, g_hi]))
    nb2 = np.ascontiguousarray((-0.5 * grid * grid).reshape(G, 1))

    in_maps = []
    for c in range(NCORES):
        xT = np.ascontiguousarray(x[c * BL:(c + 1) * BL, :].T)  # (NF, BL)
        mx2 = (-0.5 * xT * xT).ravel()
        xr = xT.ravel()
        mx2_hi, mx2_lo = _hi_lo(mx2, bf16)
        x_hi, x_lo = _hi_lo(xr, bf16)
        x5 = np.ascontiguousarray(np.stack([mx2_hi, mx2_lo, x_hi, x_hi, x_lo]))
        xs = np.ascontiguousarray(
            (xT / (1.0 + np.exp(-xT))).ravel().astype(bf16).reshape(1, NF * BL)
        )  # silu, bf16
        in_maps.append(
            {"x5": x5, "xs": xs, "wcatT": wcatT, "s5": s5, "nb2": nb2}
        )
    return in_maps


def _run(in_maps, **kw):
    from concourse.bass_utils import run_bass_kernel_spmd

    if "nc" not in _cache:
        _cache["nc"] = _build()
    return run_bass_kernel_spmd(_cache["nc"], in_maps, list(range(NCORES)), **kw)


def kernel(x, base_weight, gp_weight, grid):
    in_maps = _prep_inputs(x, base_weight, gp_weight, grid)
    res = _run(in_maps)
    return np.concatenate(
        [np.asarray(r["out"]).astype(np.float32) for r in res.results], axis=0
    )
